# revision 21
# baseline (speedup 1.0000x reference)
"""Trainium2 Bass kernel: row-wise top-k (k=32) pooling layer.

Computes, for x [500000, 128] f32 and v [128, 128] f32:
    y = (x @ v) / ||v||_F
    idx = top_k(y, 32) indices (sorted by value desc, ties -> lower index)
    out = gather(x, idx) * sigmoid(gather(y, idx))      # [500000, 32] f32

Sharding: x row-wise across 8 NeuronCores (62500 rows each, zero-padded to
123*512 = 62976 rows so every core runs an identical 512-row-block program);
v replicated (pre-scaled by 1/||v|| on host).

Per 512-row block (4 sub-tiles of 128 rows; partition p holds rows 4p+q so
both input and output DMAs are 512B-contiguous per partition):
  PE   : transpose x-subtile -> xT (PSUM), matmul(lhsT=xT, rhs=v') -> y (PSUM)
  ACT  : copy xT and y PSUM->SBUF, T-1 fixup, u16 lo/hi splits, sigmoid
  DVE  : 4x (max8 + max_index) + 3x match_replace per subtile, final multiply
  Pool : per-q slot scatters (T), exact-f32 gather via two u16 scatters
  DMA  : block in (256KB) / out (64KB)

The DVE top-k chain is the measured bottleneck (~98%% busy); all other
engines hide underneath it.
"""

import sys
from contextlib import ExitStack

import numpy as np

sys.path.insert(0, "/opt/trn_rl_repo")

import concourse.bacc as bacc
import concourse.mybir as mybir
import concourse.tile as tile
from concourse import masks
from concourse.bass_utils import run_bass_kernel_spmd

N_CORES = 8
N_FULL = 500000
D = 128
K = 32
ROWS_PER_CORE = N_FULL // N_CORES          # 62500
Q = 4                                      # sub-tiles per block
BLOCK_ROWS = 128 * Q                       # 512
N_BLOCKS = (ROWS_PER_CORE + BLOCK_ROWS - 1) // BLOCK_ROWS   # 123
ROWS_PADDED = N_BLOCKS * BLOCK_ROWS        # 62976
NEG = -1.0e30

f32 = mybir.dt.float32
i16 = mybir.dt.int16
u16 = mybir.dt.uint16


def build_kernel(n_blocks: int = N_BLOCKS):
    nc = bacc.Bacc("TRN2", target_bir_lowering=False, debug=False)

    rows = n_blocks * BLOCK_ROWS
    x_dram = nc.dram_tensor("x", [rows, D], f32, kind="ExternalInput").ap()
    v_dram = nc.dram_tensor("v", [D, D], f32, kind="ExternalInput").ap()
    out_dram = nc.dram_tensor("out", [rows, K], f32, kind="ExternalOutput").ap()

    # row (b, p, q) = 512*b + 4*p + q
    x_view = x_dram.rearrange("(b p q) d -> b p (q d)", p=128, q=Q)
    out_view = out_dram.rearrange("(b p q) j -> b p (q j)", p=128, q=Q)

    with tile.TileContext(nc) as tc, ExitStack() as ctx:
        consts = ctx.enter_context(tc.tile_pool(name="consts", bufs=1))
        xin_pool = ctx.enter_context(tc.tile_pool(name="xin", bufs=4))
        xt_pool = ctx.enter_context(tc.tile_pool(name="xt", bufs=4))
        y_pool = ctx.enter_context(tc.tile_pool(name="y", bufs=3))
        stage_pool = ctx.enter_context(tc.tile_pool(name="stage", bufs=3))
        top_pool = ctx.enter_context(tc.tile_pool(name="top", bufs=3))
        idx_pool = ctx.enter_context(tc.tile_pool(name="idx", bufs=3))
        t_pool = ctx.enter_context(tc.tile_pool(name="tarr", bufs=3))
        xg_pool = ctx.enter_context(tc.tile_pool(name="xg", bufs=3))
        out_pool = ctx.enter_context(tc.tile_pool(name="outp", bufs=3))
        psum_pool = ctx.enter_context(tc.tile_pool(name="psum", bufs=4, space="PSUM"))

        # --- constants ---
        ident = consts.tile([128, 128], f32)
        masks.make_identity(nc, ident[:])

        v_sb = consts.tile([128, D], f32)
        nc.sync.dma_start(v_sb[:], v_dram)

        # slots_plus1[p, q*K + j] = q*K + j + 1   (scatter data for T build)
        slots_plus1 = consts.tile([128, Q * K], i16)
        nc.gpsimd.iota(slots_plus1[:], pattern=[[1, Q * K]], base=1,
                       channel_multiplier=0)

        for b in range(n_blocks):
            # ---- load block: x_sb[p, q, d] = x[512b + 4p + q, d] ----
            x_sb = xin_pool.tile([128, Q, D], f32)
            nc.sync.dma_start(x_sb[:].rearrange("p q d -> p (q d)"), x_view[b])

            y_sb = y_pool.tile([128, Q, D], f32)
            y_top = top_pool.tile([128, Q * K], f32)
            idx_all = idx_pool.tile([128, Q * K], u16)

            for q in range(Q):
                # ---- matmul: y = x_q @ v' ----
                xt_psum = psum_pool.tile([128, 128], f32, tag="xt_ps")
                nc.tensor.transpose(xt_psum[:], x_sb[:, q], ident[:])
                xt_sb = xt_pool.tile([128, 128], f32)
                nc.scalar.copy(xt_sb[:], xt_psum[:])

                y_psum = psum_pool.tile([128, D], f32, tag="y_ps")
                nc.tensor.matmul(y_psum[:], xt_sb[:], v_sb[:])
                nc.scalar.copy(y_sb[:, q], y_psum[:])

                # ---- top-32 values + indices (DVE) ----
                ysb = y_sb[:, q]
                yt = y_top[:, q * K:(q + 1) * K]
                iq = idx_all[:, q * K:(q + 1) * K]
                st0 = stage_pool.tile([128, D], f32, tag="st0")
                st1 = stage_pool.tile([128, D], f32, tag="st1")
                st = [st0, st1]

                # stage r array: where rounds 0..r-1 winners are already
                # NEG-replaced.  max_index runs against the same stage its
                # round's max read, so exact-duplicate values resolve to
                # distinct indices (jax tie order).  Interleaved because
                # st[0] is reused for stage 3 (WAR handled by Tile).
                stages = [ysb, st[0][:], st[1][:], st[0][:]]
                for r in range(4):
                    nc.vector.max(yt[:, 8 * r:8 * r + 8], stages[r])
                    nc.vector.max_index(
                        iq[:, 8 * r:8 * r + 8], yt[:, 8 * r:8 * r + 8],
                        stages[r])
                    if r < 3:
                        nc.vector.match_replace(
                            stages[r + 1], yt[:, 8 * r:8 * r + 8], stages[r],
                            NEG)

            # ---- build T: T[p, q*128 + idx] = q*K + j + 1, 0 elsewhere ----
            # Per-q scatters: the data const carries the global slot value,
            # so no index-offset add is needed on DVE.
            t_arr = t_pool.tile([128, Q, D], i16)
            for q in range(Q):
                nc.gpsimd.local_scatter(
                    t_arr[:, q], slots_plus1[:, q * K:(q + 1) * K],
                    idx_all[:, q * K:(q + 1) * K].bitcast(i16),
                    channels=128, num_elems=D, num_idxs=K)
            # T-1: selected -> output slot (0..127), unselected -> -1
            t1 = t_pool.tile([128, Q * D], i16, tag="t1")
            nc.scalar.activation(t1[:], t_arr[:].rearrange("p q d -> p (q d)"),
                                 mybir.ActivationFunctionType.Copy, bias=-1.0)

            # ---- exact f32 gather of x: scatter lo/hi u16 halves by T1 ----
            x_u16 = x_sb[:].bitcast(u16)          # [128, Q, 2D] (lo,hi pairs)
            lo_sb = xg_pool.tile([128, Q * D], u16, tag="lo")
            hi_sb = xg_pool.tile([128, Q * D], u16, tag="hi")
            nc.scalar.copy(lo_sb[:].rearrange("p (q d) -> p q d", q=Q),
                           x_u16[:, :, 0::2])
            nc.scalar.copy(hi_sb[:].rearrange("p (q d) -> p q d", q=Q),
                           x_u16[:, :, 1::2])
            xg_lo = xg_pool.tile([128, Q * K], u16, tag="xglo")
            xg_hi = xg_pool.tile([128, Q * K], u16, tag="xghi")
            nc.gpsimd.local_scatter(xg_lo[:], lo_sb[:], t1[:], channels=128,
                                    num_elems=Q * K, num_idxs=Q * D)
            nc.gpsimd.local_scatter(xg_hi[:], hi_sb[:], t1[:], channels=128,
                                    num_elems=Q * K, num_idxs=Q * D)
            xg = xg_pool.tile([128, Q * K], f32, tag="xg")
            xg_u16 = xg[:].bitcast(u16)           # [128, 2*Q*K]
            nc.gpsimd.tensor_copy(xg_u16[:, 0::2], xg_lo[:])
            nc.gpsimd.tensor_copy(xg_u16[:, 1::2], xg_hi[:])

            # ---- out = xg * sigmoid(y_top) ----
            sig = top_pool.tile([128, Q * K], f32, tag="sig")
            nc.scalar.activation(sig[:], y_top[:],
                                 mybir.ActivationFunctionType.Sigmoid)
            out_sb = out_pool.tile([128, Q * K], f32)
            nc.vector.tensor_tensor(out_sb[:], xg[:], sig[:],
                                    mybir.AluOpType.mult)

            nc.sync.dma_start(out_view[b], out_sb[:])

    nc.compile()
    return nc


_nc_cache = {}


def kernel(x: np.ndarray, v: np.ndarray, k):
    assert int(k) == K
    x = np.asarray(x, dtype=np.float32)
    v = np.asarray(v, dtype=np.float32)
    assert x.shape == (N_FULL, D) and v.shape == (D, D)

    length = np.linalg.norm(v)
    v_scaled = (v / length).astype(np.float32)

    if "nc" not in _nc_cache:
        _nc_cache["nc"] = build_kernel()
    nc = _nc_cache["nc"]

    pad = np.zeros((ROWS_PADDED - ROWS_PER_CORE, D), dtype=np.float32)
    in_maps = []
    for c in range(N_CORES):
        shard = x[c * ROWS_PER_CORE:(c + 1) * ROWS_PER_CORE]
        in_maps.append({"x": np.concatenate([shard, pad], axis=0),
                        "v": v_scaled})

    res = run_bass_kernel_spmd(nc, in_maps, list(range(N_CORES)))
    out = np.concatenate(
        [res.results[c]["out"][:ROWS_PER_CORE] for c in range(N_CORES)], axis=0)
    return out.astype(np.float32)


# revision 23
# speedup vs baseline: 1.3498x; 1.3498x over previous
"""Trainium2 Bass kernel: row-wise top-k (k=32) pooling layer.

Computes, for x [500000, 128] f32 and v [128, 128] f32:
    y = (x @ v) / ||v||_F
    idx = top_k(y, 32) indices (sorted by value desc, ties -> lower index)
    out = gather(x, idx) * sigmoid(gather(y, idx))      # [500000, 32] f32

Sharding: x row-wise across 8 NeuronCores (62500 rows each, zero-padded to
123*512 = 62976 rows so every core runs an identical 512-row-block program);
v replicated (pre-scaled by 1/||v|| on host).

Per 512-row block (4 sub-tiles of 128 rows; partition p holds rows 4p+q so
both input and output DMAs are 512B-contiguous per partition):
  PE   : transpose x-subtile -> xT (PSUM), matmul(lhsT=xT, rhs=v') -> y (PSUM)
  ACT  : copy xT and y PSUM->SBUF, T-1 fixup, u16 lo/hi splits, sigmoid
  DVE  : 4x (max8 + max_index) + 3x match_replace per subtile, final multiply
  Pool : per-q slot scatters (T), exact-f32 gather via two u16 scatters
  DMA  : block in (256KB) / out (64KB)

The DVE top-k chain is the measured bottleneck (~98%% busy); all other
engines hide underneath it.
"""

import sys
from contextlib import ExitStack

import numpy as np

sys.path.insert(0, "/opt/trn_rl_repo")

import concourse.bacc as bacc
import concourse.mybir as mybir
import concourse.tile as tile
from concourse import masks
from concourse.bass_utils import run_bass_kernel_spmd

N_CORES = 8
N_FULL = 500000
D = 128
K = 32
ROWS_PER_CORE = N_FULL // N_CORES          # 62500
Q = 4                                      # sub-tiles per block
BLOCK_ROWS = 128 * Q                       # 512
N_BLOCKS = (ROWS_PER_CORE + BLOCK_ROWS - 1) // BLOCK_ROWS   # 123
ROWS_PADDED = N_BLOCKS * BLOCK_ROWS        # 62976
NEG = -1.0e30

f32 = mybir.dt.float32
i16 = mybir.dt.int16
u16 = mybir.dt.uint16


def build_kernel(n_blocks: int = N_BLOCKS):
    nc = bacc.Bacc("TRN2", target_bir_lowering=False, debug=False)

    rows = n_blocks * BLOCK_ROWS
    x_dram = nc.dram_tensor("x", [rows, D], f32, kind="ExternalInput").ap()
    v_dram = nc.dram_tensor("v", [D, D], f32, kind="ExternalInput").ap()
    out_dram = nc.dram_tensor("out", [rows, K], f32, kind="ExternalOutput").ap()

    # row (b, p, q) = 512*b + 4*p + q
    x_view = x_dram.rearrange("(b p q) d -> b p (q d)", p=128, q=Q)
    out_view = out_dram.rearrange("(b p q) j -> b p (q j)", p=128, q=Q)

    with tile.TileContext(nc) as tc, ExitStack() as ctx:
        consts = ctx.enter_context(tc.tile_pool(name="consts", bufs=1))
        xin_pool = ctx.enter_context(tc.tile_pool(name="xin", bufs=4))
        xt_pool = ctx.enter_context(tc.tile_pool(name="xt", bufs=4))
        y_pool = ctx.enter_context(tc.tile_pool(name="y", bufs=3))
        stage_pool = ctx.enter_context(tc.tile_pool(name="stage", bufs=3))
        top_pool = ctx.enter_context(tc.tile_pool(name="top", bufs=3))
        idx_pool = ctx.enter_context(tc.tile_pool(name="idx", bufs=3))
        t_pool = ctx.enter_context(tc.tile_pool(name="tarr", bufs=3))
        xg_pool = ctx.enter_context(tc.tile_pool(name="xg", bufs=3))
        out_pool = ctx.enter_context(tc.tile_pool(name="outp", bufs=3))
        psum_pool = ctx.enter_context(tc.tile_pool(name="psum", bufs=4, space="PSUM"))

        # --- constants ---
        ident = consts.tile([128, 128], f32)
        masks.make_identity(nc, ident[:])

        v_sb = consts.tile([128, D], f32)
        nc.sync.dma_start(v_sb[:], v_dram)

        # slots_plus1[p, q*K + j] = q*K + j + 1   (scatter data for T build)
        slots_plus1 = consts.tile([128, Q * K], i16)
        nc.gpsimd.iota(slots_plus1[:], pattern=[[1, Q * K]], base=1,
                       channel_multiplier=0)

        pending = None
        for b in range(n_blocks):
            # ---- load block: x_sb[p, q, d] = x[512b + 4p + q, d] ----
            x_sb = xin_pool.tile([128, Q, D], f32)
            nc.sync.dma_start(x_sb[:].rearrange("p q d -> p (q d)"), x_view[b])

            y_sb = y_pool.tile([128, Q, D], f32)
            y_top = top_pool.tile([128, Q * K], f32)
            idx_all = idx_pool.tile([128, Q * K], u16)

            for q in range(Q):
                # ---- matmul: y = x_q @ v' ----
                xt_psum = psum_pool.tile([128, 128], f32, tag="xt_ps")
                nc.tensor.transpose(xt_psum[:], x_sb[:, q], ident[:])
                xt_sb = xt_pool.tile([128, 128], f32)
                nc.scalar.copy(xt_sb[:], xt_psum[:])

                y_psum = psum_pool.tile([128, D], f32, tag="y_ps")
                nc.tensor.matmul(y_psum[:], xt_sb[:], v_sb[:])
                nc.scalar.copy(y_sb[:, q], y_psum[:])

                # ---- top-32 values + indices (DVE) ----
                ysb = y_sb[:, q]
                yt = y_top[:, q * K:(q + 1) * K]
                iq = idx_all[:, q * K:(q + 1) * K]
                st0 = stage_pool.tile([128, D], f32, tag="st0")
                st1 = stage_pool.tile([128, D], f32, tag="st1")
                st = [st0, st1]

                # stage r array: where rounds 0..r-1 winners are already
                # NEG-replaced.  max_index runs against the same stage its
                # round's max read, so exact-duplicate values resolve to
                # distinct indices (jax tie order).  Interleaved because
                # st[0] is reused for stage 3 (WAR handled by Tile).
                stages = [ysb, st[0][:], st[1][:], st[0][:]]
                for r in range(4):
                    nc.vector.max(yt[:, 8 * r:8 * r + 8], stages[r])
                    nc.vector.max_index(
                        iq[:, 8 * r:8 * r + 8], yt[:, 8 * r:8 * r + 8],
                        stages[r])
                    if r < 3:
                        nc.vector.match_replace(
                            stages[r + 1], yt[:, 8 * r:8 * r + 8], stages[r],
                            NEG)

            # ---- build T: T[p, q*128 + idx] = q*K + j + 1, 0 elsewhere ----
            # Per-q scatters: the data const carries the global slot value,
            # so no index-offset add is needed on DVE.
            t_arr = t_pool.tile([128, Q, D], i16)
            for q in range(Q):
                nc.gpsimd.local_scatter(
                    t_arr[:, q], slots_plus1[:, q * K:(q + 1) * K],
                    idx_all[:, q * K:(q + 1) * K].bitcast(i16),
                    channels=128, num_elems=D, num_idxs=K)
            # T-1: selected -> output slot (0..127), unselected -> -1
            t1 = t_pool.tile([128, Q * D], i16, tag="t1")
            nc.scalar.activation(t1[:], t_arr[:].rearrange("p q d -> p (q d)"),
                                 mybir.ActivationFunctionType.Copy, bias=-1.0)

            # ---- exact f32 gather of x: scatter lo/hi u16 halves by T1 ----
            x_u16 = x_sb[:].bitcast(u16)          # [128, Q, 2D] (lo,hi pairs)
            lo_sb = xg_pool.tile([128, Q * D], u16, tag="lo")
            hi_sb = xg_pool.tile([128, Q * D], u16, tag="hi")
            nc.scalar.copy(lo_sb[:].rearrange("p (q d) -> p q d", q=Q),
                           x_u16[:, :, 0::2])
            nc.scalar.copy(hi_sb[:].rearrange("p (q d) -> p q d", q=Q),
                           x_u16[:, :, 1::2])
            xg_lo = xg_pool.tile([128, Q * K], u16, tag="xglo")
            xg_hi = xg_pool.tile([128, Q * K], u16, tag="xghi")
            nc.gpsimd.local_scatter(xg_lo[:], lo_sb[:], t1[:], channels=128,
                                    num_elems=Q * K, num_idxs=Q * D)
            nc.gpsimd.local_scatter(xg_hi[:], hi_sb[:], t1[:], channels=128,
                                    num_elems=Q * K, num_idxs=Q * D)
            xg = xg_pool.tile([128, Q * K], f32, tag="xg")
            xg_u16 = xg[:].bitcast(u16)           # [128, 2*Q*K]
            nc.gpsimd.tensor_copy(xg_u16[:, 0::2], xg_lo[:])
            nc.gpsimd.tensor_copy(xg_u16[:, 1::2], xg_hi[:])

            # ---- out = xg * sigmoid(y_top) ----
            sig = top_pool.tile([128, Q * K], f32, tag="sig")
            nc.scalar.activation(sig[:], y_top[:],
                                 mybir.ActivationFunctionType.Sigmoid)

            # Software-pipeline the DVE multiply: emit the PREVIOUS block's
            # multiply here so it sits after this block's top-k in the
            # in-order DVE stream — by then its xg is long ready.  Emitting
            # it in its own block makes the DVE stall ~3.6us/block waiting
            # for the Pool gather chain (measured 450us total).
            if pending is not None:
                pxg, psig, pb = pending
                out_sb = out_pool.tile([128, Q * K], f32, tag="out_sb")
                nc.vector.tensor_tensor(out_sb[:], pxg[:], psig[:],
                                        mybir.AluOpType.mult)
                nc.sync.dma_start(out_view[pb], out_sb[:])
            pending = (xg, sig, b)

        pxg, psig, pb = pending
        out_sb = out_pool.tile([128, Q * K], f32, tag="out_sb")
        nc.vector.tensor_tensor(out_sb[:], pxg[:], psig[:],
                                mybir.AluOpType.mult)
        nc.sync.dma_start(out_view[pb], out_sb[:])

    nc.compile()
    return nc


_nc_cache = {}


def kernel(x: np.ndarray, v: np.ndarray, k):
    assert int(k) == K
    x = np.asarray(x, dtype=np.float32)
    v = np.asarray(v, dtype=np.float32)
    assert x.shape == (N_FULL, D) and v.shape == (D, D)

    length = np.linalg.norm(v)
    v_scaled = (v / length).astype(np.float32)

    if "nc" not in _nc_cache:
        _nc_cache["nc"] = build_kernel()
    nc = _nc_cache["nc"]

    pad = np.zeros((ROWS_PADDED - ROWS_PER_CORE, D), dtype=np.float32)
    in_maps = []
    for c in range(N_CORES):
        shard = x[c * ROWS_PER_CORE:(c + 1) * ROWS_PER_CORE]
        in_maps.append({"x": np.concatenate([shard, pad], axis=0),
                        "v": v_scaled})

    res = run_bass_kernel_spmd(nc, in_maps, list(range(N_CORES)))
    out = np.concatenate(
        [res.results[c]["out"][:ROWS_PER_CORE] for c in range(N_CORES)], axis=0)
    return out.astype(np.float32)
